# revision 2
# baseline (speedup 1.0000x reference)
"""Ternary (BitwiseLinear) matmul kernel for Trainium2, 8-core data-parallel.

y = ternary(x) @ ternary(w).T  with threshold 0.05, int-exact accumulation.

Sharding: x is split along the token dim across 8 cores (4096 tokens each);
the weight is replicated. Each core computes its y shard independently
(no collectives) and shards are concatenated on the host.

Per-core pipeline (v3):
  1. quantize to ternary: u = (v>=T), vneg = (v<=-T), q = u - vneg on DVE
     -> bf16 {-1,0,1}. Exact (no element == +-T).
  2. PE-transpose q 128x128 blocks into one bf16 PSUM bank; evict with a
     single copy per tile, casting to fp8e4 -> k-major layout.
  3. fp8 DoubleRow matmuls (K=256 per instruction) accumulate y tile
     [t:128, o:2x512] in a 2-bank PSUM tile; evict to fp16 (exact: |y| <=
     1024 < 2048), DMA out. Host upcasts fp16 -> f32.
  4. x loads on the SP HWDGE ring, w loads + y stores on the ACT ring so
     input and output streams don't serialize on one ring.
"""

import threading

import numpy as np

N_CORES = 8
TOKENS = 32768
TOK_PER_CORE = TOKENS // N_CORES
K = 1024
O = 1024
P = 128
THR = 0.05

_cache = {}
_lock = threading.Lock()


def _split_multi_waits(nc):
    """walrus in this env can't encode >1 sync wait on one instruction: hoist
    extra waits into single-wait NOPs on the same engine, just before the
    instruction (identical per-engine wait semantics)."""
    import concourse.mybir as mybir

    uid = 0
    for f in nc.m.functions:
        for b in f.blocks:
            out = []
            changed = False
            for inst in b.instructions:
                si = inst.sync_info
                if si is not None and si.on_wait and len(si.on_wait) > 1:
                    waits = list(si.on_wait)
                    for w in waits[:-1]:
                        uid += 1
                        out.append(mybir.InstNoOp(
                            name=f"I-waitsplit-{uid}",
                            engine=inst.engine,
                            sync_info=mybir.SyncInfo(on_wait=[w], on_update=[]),
                        ))
                    inst.sync_info = mybir.SyncInfo(
                        on_wait=[waits[-1]], on_update=list(si.on_update))
                    changed = True
                out.append(inst)
            if changed:
                b.instructions = out
    return nc


def build_nc(tokens=TOK_PER_CORE, loop_n=1, *,
             out_fp16=True,       # y stored as fp16 (exact; host upcasts)
             x_dma="sync",        # HWDGE ring for x loads
             w_dma="scalar",      # ring for w loads
             y_dma="scalar",      # ring for y stores
             tevict="vector",     # engine for psum_t (xqT/wqT) evictions
             yevict="scalar",     # engine for psum_y evictions
             ldw_share=True,      # s-outer/oh-inner matmul order
             xbatch=2,            # token tiles per x load DMA
             quant_ops=3,         # ablation: 1 = single DVE op (wrong result)
             do_transpose=True,   # ablation: False = evict from q directly
             do_mm=True,          # ablation: False = ysb evicted from q
             ):
    import concourse.bass as bass
    import concourse.mybir as mybir
    from concourse.masks import make_identity
    from concourse.tile import TileContext

    F32 = mybir.dt.float32
    F16 = mybir.dt.float16
    BF16 = mybir.dt.bfloat16
    FP8 = mybir.dt.float8e4
    A = mybir.AluOpType
    YDT = F16 if out_fp16 else F32

    KB = K // P          # 8 k-blocks of 128
    n_ttiles = tokens // P

    nc = bass.Bass()
    x = nc.dram_tensor("x", [tokens, K], F32, kind="ExternalInput")
    w = nc.dram_tensor("weight", [O, K], F32, kind="ExternalInput")
    y = nc.dram_tensor("out", [tokens, O], YDT, kind="ExternalOutput")

    # [t, k] viewed as [t-block, p, k] for batched loads
    x2 = x.rearrange("(a p) k -> a p k", p=P)
    w2 = w.rearrange("(a p) k -> a p k", p=P)
    y2 = y.rearrange("(a p) k -> a p k", p=P)

    def eng(name):
        return {"sync": nc.sync, "scalar": nc.scalar, "vector": nc.vector,
                "gpsimd": nc.gpsimd}[name]

    with TileContext(nc) as tc:
        with (
            tc.tile_pool(name="const", bufs=1) as const_pool,
            tc.tile_pool(name="wqt", bufs=1) as wqt_pool,
            tc.tile_pool(name="win", bufs=2) as win_pool,
            tc.tile_pool(name="xin", bufs=3) as xin_pool,
            tc.tile_pool(name="quant", bufs=3) as q_pool,
            tc.tile_pool(name="xqt", bufs=3) as xqt_pool,
            tc.tile_pool(name="yout", bufs=3) as y_pool,
            tc.tile_pool(name="psum_t", bufs=2, space="PSUM") as psumt_pool,
            tc.tile_pool(name="psum_y", bufs=3, space="PSUM") as psumy_pool,
        ):
            identity = const_pool.tile([P, P], BF16)
            make_identity(nc, identity)

            def quantize(src):
                """f32 [128, K] view -> ternary bf16 [128, K]."""
                q = q_pool.tile([P, K], BF16, tag="q_q")
                if quant_ops == 1:
                    nc.vector.tensor_scalar(
                        out=q[:], in0=src, scalar1=THR, scalar2=None,
                        op0=A.is_ge)
                    return q
                u = q_pool.tile([P, K], BF16, tag="q_u")
                nc.vector.tensor_scalar(
                    out=u[:], in0=src, scalar1=THR, scalar2=None, op0=A.is_ge)
                v = q_pool.tile([P, K], BF16, tag="q_v")
                nc.vector.tensor_scalar(
                    out=v[:], in0=src, scalar1=-THR, scalar2=None, op0=A.is_le)
                nc.vector.tensor_tensor(out=q[:], in0=u[:], in1=v[:],
                                        op=A.subtract)
                return q

            def transpose_to(q, dst, evict_engine):
                """q bf16 [128, K] natural -> dst fp8 [128, KB, 128] k-major."""
                if not do_transpose:
                    src = q[:].rearrange("p (b c) -> p b c", b=KB)
                    if evict_engine == "vector":
                        nc.vector.tensor_copy(dst[:], src)
                    else:
                        nc.scalar.copy(dst[:], src)
                    return
                ps = psumt_pool.tile([P, KB, P], BF16, tag="psT")
                for kb in range(KB):
                    nc.tensor.transpose(
                        ps[:, kb, :], q[:, kb * P:(kb + 1) * P], identity)
                if evict_engine == "vector":
                    nc.vector.tensor_copy(dst[:], ps[:])
                else:
                    nc.scalar.copy(dst[:], ps[:])

            # --- weight phase: wqT fp8 [k_part, k_blk, o] ---
            wqT = wqt_pool.tile([P, KB, O], FP8)
            for pair in range(O // (2 * P)):       # 4 batched loads
                wt = win_pool.tile([P, 2, K], F32, tag="w_in")
                eng(w_dma).dma_start(
                    wt[:], w2[2 * pair:2 * pair + 2].rearrange("a p k -> p a k"))
                for j in range(2):
                    ob = 2 * pair + j
                    qw = quantize(wt[:, j, :])
                    transpose_to(qw, wqT[:, :, ob * P:(ob + 1) * P], "scalar")

            def do_tile(tb, qx, ysb, j):
                """compute y tile tb from quantized qx, write into ysb[:, j]."""
                xqT = xqt_pool.tile([P, KB, P], FP8, tag="xqT")
                transpose_to(qx, xqT, tevict)

                ydst = ysb[:, j, :]
                if not do_mm:
                    src = qx[:]
                    if yevict == "vector":
                        nc.vector.tensor_copy(ydst, src)
                    else:
                        nc.scalar.copy(ydst, src)
                    return
                yp = psumy_pool.tile([P, 2, 512], F32, tag="yp")
                if ldw_share:
                    for s in range(KB // 2):       # 4 DoubleRow k-steps
                        for oh in range(2):
                            nc.tensor.matmul(
                                yp[:, oh, :],
                                xqT[:, 2 * s:2 * s + 2, :],
                                wqT[:, 2 * s:2 * s + 2,
                                    oh * 512:(oh + 1) * 512],
                                start=(s == 0),
                                stop=(s == KB // 2 - 1),
                                perf_mode=mybir.MatmulPerfMode.DoubleRow,
                            )
                else:
                    for oh in range(2):
                        for s in range(KB // 2):
                            nc.tensor.matmul(
                                yp[:, oh, :],
                                xqT[:, 2 * s:2 * s + 2, :],
                                wqT[:, 2 * s:2 * s + 2,
                                    oh * 512:(oh + 1) * 512],
                                start=(s == 0),
                                stop=(s == KB // 2 - 1),
                                perf_mode=mybir.MatmulPerfMode.DoubleRow,
                            )
                src = yp[:].rearrange("p a b -> p (a b)")
                if yevict == "vector":
                    nc.vector.tensor_copy(ydst, src)
                else:
                    nc.scalar.copy(ydst, src)

            def main_body():
                for tp in range(n_ttiles // xbatch):
                    xt = xin_pool.tile([P, xbatch, K], F32, tag="x_in")
                    eng(x_dma).dma_start(
                        xt[:],
                        x2[xbatch * tp:xbatch * (tp + 1)]
                        .rearrange("a p k -> p a k"))
                    ysb = y_pool.tile([P, xbatch, O], YDT, tag="ysb")
                    for j in range(xbatch):
                        tb = xbatch * tp + j
                        qx = quantize(xt[:, j, :])
                        do_tile(tb, qx, ysb, j)
                    eng(y_dma).dma_start(
                        y2[xbatch * tp:xbatch * (tp + 1)]
                        .rearrange("a p k -> p a k"),
                        ysb[:])

            # loop_n > 1 wraps the token loop in a hardware loop purely for
            # benchmarking (amortizes per-call host/PJRT overhead).
            if loop_n > 1:
                with tc.For_i(0, loop_n, 1):
                    main_body()
            else:
                main_body()

    _split_multi_waits(nc)
    return nc


def _get_nc(tokens=TOK_PER_CORE):
    with _lock:
        if tokens not in _cache:
            _cache[tokens] = build_nc(tokens)
        return _cache[tokens]


def kernel(x: np.ndarray, weight: np.ndarray):
    from concourse.bass_utils import run_bass_kernel_spmd

    x = np.ascontiguousarray(x, dtype=np.float32)
    weight = np.ascontiguousarray(weight, dtype=np.float32)
    assert x.shape == (TOKENS, K) and weight.shape == (O, K)

    nc = _get_nc()
    in_maps = [
        {"x": x[i * TOK_PER_CORE:(i + 1) * TOK_PER_CORE], "weight": weight}
        for i in range(N_CORES)
    ]
    res = run_bass_kernel_spmd(nc, in_maps, core_ids=list(range(N_CORES)))
    return np.concatenate(
        [r["out"].astype(np.float32) for r in res.results], axis=0)


# revision 5
# speedup vs baseline: 1.0782x; 1.0782x over previous
"""Ternary (BitwiseLinear) matmul kernel for Trainium2, 8-core data-parallel.

y = ternary(x) @ ternary(w).T  with threshold 0.05, int-exact accumulation.

Sharding: x is split along the token dim across 8 cores (4096 tokens each);
the weight is replicated. Each core computes its y shard independently
(no collectives) and shards are concatenated on the host.

Per-core pipeline (v4):
  1. quantize to NEGATED ternary in 2 ops: u = (v>=T) on GPSIMD, then
     qneg = (v<=-T) - u on DVE (fused scalar_tensor_tensor) -> bf16
     {-1,0,1} = -ternary(v). Negation applied to BOTH x and w, so the
     matmul product is unchanged. Exact (no element == +-T).
  2. PE-transpose qneg 128x128 blocks into one bf16 PSUM bank; evict on
     ACT with a single copy per tile, casting to fp8e4 -> k-major layout.
  3. fp8 DoubleRow matmuls (K=256 per instruction) accumulate y tile
     [t:128, o:2x512] in a 2-bank PSUM tile; evict on ACT to fp16 (exact:
     |y| <= 1024 < 2048), DMA out. Host upcasts fp16 -> f32.
  4. x loads on the SP HWDGE ring, w loads + y stores on the ACT ring so
     input and output streams don't serialize on one ring.

Engine budget per loop iteration (32 token tiles): DMA ~83us (24 MiB),
ACT evicts ~55us, DVE ~25us, GPSIMD ~28us, PE transposes+matmuls.
"""

import threading

import numpy as np

N_CORES = 8
TOKENS = 32768
TOK_PER_CORE = TOKENS // N_CORES
K = 1024
O = 1024
P = 128
THR = 0.05

_cache = {}
_lock = threading.Lock()


def _split_multi_waits(nc):
    """walrus in this env can't encode >1 sync wait on one instruction: hoist
    extra waits into single-wait NOPs on the same engine, just before the
    instruction (identical per-engine wait semantics)."""
    import concourse.mybir as mybir

    uid = 0
    for f in nc.m.functions:
        for b in f.blocks:
            out = []
            changed = False
            for inst in b.instructions:
                si = inst.sync_info
                if si is not None and si.on_wait and len(si.on_wait) > 1:
                    waits = list(si.on_wait)
                    for w in waits[:-1]:
                        uid += 1
                        out.append(mybir.InstNoOp(
                            name=f"I-waitsplit-{uid}",
                            engine=inst.engine,
                            sync_info=mybir.SyncInfo(on_wait=[w], on_update=[]),
                        ))
                    inst.sync_info = mybir.SyncInfo(
                        on_wait=[waits[-1]], on_update=list(si.on_update))
                    changed = True
                out.append(inst)
            if changed:
                b.instructions = out
    return nc


def build_nc(tokens=TOK_PER_CORE, loop_n=1, *,
             out_fp16=True,       # y stored as fp16 (exact; host upcasts)
             x_dma="sync",        # HWDGE ring for x loads
             w_dma="scalar",      # ring for w loads
             y_dma="scalar",      # ring for y stores
             tevict="scalar",     # engine for psum_t (xqT/wqT) evictions
             yevict="scalar",     # engine for psum_y evictions
             ldw_share=True,      # s-outer/oh-inner matmul order
             xbatch=2,            # token tiles per x load DMA
             quant_mode="split2", # split2 | dve2 | dve3 | dve1 (ablation)
             do_transpose=True,   # ablation: False = evict from q directly
             do_mm=True,          # ablation: False = ysb evicted from q
             ):
    import concourse.bass as bass
    import concourse.mybir as mybir
    from concourse.masks import make_identity
    from concourse.tile import TileContext

    F32 = mybir.dt.float32
    F16 = mybir.dt.float16
    BF16 = mybir.dt.bfloat16
    FP8 = mybir.dt.float8e4
    A = mybir.AluOpType
    YDT = F16 if out_fp16 else F32

    KB = K // P          # 8 k-blocks of 128
    n_ttiles = tokens // P

    nc = bass.Bass()
    x = nc.dram_tensor("x", [tokens, K], F32, kind="ExternalInput")
    w = nc.dram_tensor("weight", [O, K], F32, kind="ExternalInput")
    y = nc.dram_tensor("out", [tokens, O], YDT, kind="ExternalOutput")

    # [t, k] viewed as [t-block, p, k] for batched loads
    x2 = x.rearrange("(a p) k -> a p k", p=P)
    w2 = w.rearrange("(a p) k -> a p k", p=P)
    y2 = y.rearrange("(a p) k -> a p k", p=P)

    def eng(name):
        return {"sync": nc.sync, "scalar": nc.scalar, "vector": nc.vector,
                "gpsimd": nc.gpsimd}[name]

    with TileContext(nc) as tc:
        with (
            tc.tile_pool(name="const", bufs=1) as const_pool,
            tc.tile_pool(name="wqt", bufs=1) as wqt_pool,
            tc.tile_pool(name="win", bufs=2) as win_pool,
            tc.tile_pool(name="xin", bufs=3) as xin_pool,
            tc.tile_pool(name="quant", bufs=3) as q_pool,
            tc.tile_pool(name="xqt", bufs=3) as xqt_pool,
            tc.tile_pool(name="yout", bufs=3) as y_pool,
            tc.tile_pool(name="psum_t", bufs=2, space="PSUM") as psumt_pool,
            tc.tile_pool(name="psum_y", bufs=3, space="PSUM") as psumy_pool,
        ):
            identity = const_pool.tile([P, P], BF16)
            make_identity(nc, identity)

            def quantize(src):
                """f32 [128, K] view -> NEGATED ternary bf16 [128, K]."""
                q = q_pool.tile([P, K], BF16, tag="q_q")
                if quant_mode == "dve1":      # ablation only (wrong result)
                    nc.vector.tensor_scalar(
                        out=q[:], in0=src, scalar1=THR, scalar2=None,
                        op0=A.is_ge)
                    return q
                if quant_mode in ("split2", "dve2"):
                    u = q_pool.tile([P, K], BF16, tag="q_u")
                    ueng = nc.gpsimd if quant_mode == "split2" else nc.vector
                    ueng.tensor_scalar(
                        out=u[:], in0=src, scalar1=THR, scalar2=None,
                        op0=A.is_ge)
                    # q = (src <= -T) - u  ==  -ternary(src)
                    nc.vector.scalar_tensor_tensor(
                        out=q[:], in0=src, scalar=-THR, in1=u[:],
                        op0=A.is_le, op1=A.subtract)
                    return q
                # dve3: q = -(u - v) = v - u, all on DVE
                u = q_pool.tile([P, K], BF16, tag="q_u")
                nc.vector.tensor_scalar(
                    out=u[:], in0=src, scalar1=THR, scalar2=None, op0=A.is_ge)
                v = q_pool.tile([P, K], BF16, tag="q_v")
                nc.vector.tensor_scalar(
                    out=v[:], in0=src, scalar1=-THR, scalar2=None, op0=A.is_le)
                nc.vector.tensor_tensor(out=q[:], in0=v[:], in1=u[:],
                                        op=A.subtract)
                return q

            def transpose_to(q, dst, evict_engine):
                """q bf16 [128, K] natural -> dst fp8 [128, KB, 128] k-major."""
                if not do_transpose:
                    src = q[:].rearrange("p (b c) -> p b c", b=KB)
                    if evict_engine == "vector":
                        nc.vector.tensor_copy(dst[:], src)
                    else:
                        nc.scalar.copy(dst[:], src)
                    return
                ps = psumt_pool.tile([P, KB, P], BF16, tag="psT")
                for kb in range(KB):
                    nc.tensor.transpose(
                        ps[:, kb, :], q[:, kb * P:(kb + 1) * P], identity)
                if evict_engine == "vector":
                    nc.vector.tensor_copy(dst[:], ps[:])
                else:
                    nc.scalar.copy(dst[:], ps[:])

            # --- weight phase: wqT fp8 [k_part, k_blk, o] ---
            wqT = wqt_pool.tile([P, KB, O], FP8)
            for pair in range(O // (2 * P)):       # 4 batched loads
                wt = win_pool.tile([P, 2, K], F32, tag="w_in")
                eng(w_dma).dma_start(
                    wt[:], w2[2 * pair:2 * pair + 2].rearrange("a p k -> p a k"))
                for j in range(2):
                    ob = 2 * pair + j
                    qw = quantize(wt[:, j, :])
                    transpose_to(qw, wqT[:, :, ob * P:(ob + 1) * P], "scalar")

            def do_tile(tb, qx, ysb, j):
                """compute y tile tb from quantized qx, write into ysb[:, j]."""
                xqT = xqt_pool.tile([P, KB, P], FP8, tag="xqT")
                transpose_to(qx, xqT, tevict)

                ydst = ysb[:, j, :]
                if not do_mm:
                    src = qx[:]
                    if yevict == "vector":
                        nc.vector.tensor_copy(ydst, src)
                    else:
                        nc.scalar.copy(ydst, src)
                    return
                yp = psumy_pool.tile([P, 2, 512], F32, tag="yp")
                if ldw_share:
                    for s in range(KB // 2):       # 4 DoubleRow k-steps
                        for oh in range(2):
                            nc.tensor.matmul(
                                yp[:, oh, :],
                                xqT[:, 2 * s:2 * s + 2, :],
                                wqT[:, 2 * s:2 * s + 2,
                                    oh * 512:(oh + 1) * 512],
                                start=(s == 0),
                                stop=(s == KB // 2 - 1),
                                perf_mode=mybir.MatmulPerfMode.DoubleRow,
                            )
                else:
                    for oh in range(2):
                        for s in range(KB // 2):
                            nc.tensor.matmul(
                                yp[:, oh, :],
                                xqT[:, 2 * s:2 * s + 2, :],
                                wqT[:, 2 * s:2 * s + 2,
                                    oh * 512:(oh + 1) * 512],
                                start=(s == 0),
                                stop=(s == KB // 2 - 1),
                                perf_mode=mybir.MatmulPerfMode.DoubleRow,
                            )
                src = yp[:].rearrange("p a b -> p (a b)")
                if yevict == "vector":
                    nc.vector.tensor_copy(ydst, src)
                else:
                    nc.scalar.copy(ydst, src)

            def main_body():
                for tp in range(n_ttiles // xbatch):
                    xt = xin_pool.tile([P, xbatch, K], F32, tag="x_in")
                    eng(x_dma).dma_start(
                        xt[:],
                        x2[xbatch * tp:xbatch * (tp + 1)]
                        .rearrange("a p k -> p a k"))
                    ysb = y_pool.tile([P, xbatch, O], YDT, tag="ysb")
                    for j in range(xbatch):
                        tb = xbatch * tp + j
                        qx = quantize(xt[:, j, :])
                        do_tile(tb, qx, ysb, j)
                    eng(y_dma).dma_start(
                        y2[xbatch * tp:xbatch * (tp + 1)]
                        .rearrange("a p k -> p a k"),
                        ysb[:])

            # loop_n > 1 wraps the token loop in a hardware loop purely for
            # benchmarking (amortizes per-call host/PJRT overhead).
            if loop_n > 1:
                with tc.For_i(0, loop_n, 1):
                    main_body()
            else:
                main_body()

    _split_multi_waits(nc)
    return nc


def _get_nc(tokens=TOK_PER_CORE):
    with _lock:
        if tokens not in _cache:
            _cache[tokens] = build_nc(tokens)
        return _cache[tokens]


def kernel(x: np.ndarray, weight: np.ndarray):
    from concourse.bass_utils import run_bass_kernel_spmd

    x = np.ascontiguousarray(x, dtype=np.float32)
    weight = np.ascontiguousarray(weight, dtype=np.float32)
    assert x.shape == (TOKENS, K) and weight.shape == (O, K)

    nc = _get_nc()
    in_maps = [
        {"x": x[i * TOK_PER_CORE:(i + 1) * TOK_PER_CORE], "weight": weight}
        for i in range(N_CORES)
    ]
    res = run_bass_kernel_spmd(nc, in_maps, core_ids=list(range(N_CORES)))
    return np.concatenate(
        [r["out"].astype(np.float32) for r in res.results], axis=0)


# revision 21
# speedup vs baseline: 1.2161x; 1.1279x over previous
"""Ternary (BitwiseLinear) matmul kernel for Trainium2, 8-core data-parallel.

y = ternary(x) @ ternary(w).T  with threshold 0.05, int-exact accumulation.

Sharding: x is split along the token dim across 8 cores (4096 tokens each);
the weight is replicated. Each core computes its y shard independently
(no collectives) and shards are concatenated on the host.

Per-core pipeline (v6):
  1. quantize to NEGATED ternary in 2 DVE ops: u = (v>=T), then
     qneg = (v<=-T) - u (fused scalar_tensor_tensor) -> bf16 {-1,0,1}
     = -ternary(v). Negation applied to BOTH x and w, so the matmul
     product is unchanged. Exact (no element == +-T).
  2. PE-transpose qneg 128x128 blocks into one bf16 PSUM bank; evict
     split across ACT and DVE, casting to fp8e4 -> k-major layout.
  3. fp8 DoubleRow matmuls (K=256 per instruction, s-outer order with
     both 512-wide output halves live in 1-bank PSUM tiles; redundant
     Ldweights NOPed by _dedup_ldweights) accumulate y tile; evict on
     ACT to fp16 (exact: |y| <= 1024 < 2048), DMA out. Host upcasts.
  4. x loads AND y stores on the SP HWDGE ring (SP engine has no compute
     so ring serialization is free); w loads on the ACT ring. y stores
     via ACT/GPSIMD rings measured slower (ACT stalls evicts; SWDGE hits
     a walrus "ISA wrong length" bug inside hardware loops).

Measured loop steady state ~101-110us/iter (32 token tiles; run-to-run
drift +-5us): PE-paced (matmul stream 62us + transposes ~12us + visible
Ldweights/stalls), DMA 24 MiB/iter ~83us, ACT evicts ~47us, DVE ~60us.
"""

import threading

import numpy as np

N_CORES = 8
TOKENS = 32768
TOK_PER_CORE = TOKENS // N_CORES
K = 1024
O = 1024
P = 128
THR = 0.05

_cache = {}
_lock = threading.Lock()


def _split_multi_waits(nc):
    """walrus in this env can't encode >1 sync wait on one instruction: hoist
    extra waits into single-wait NOPs on the same engine, just before the
    instruction (identical per-engine wait semantics)."""
    import concourse.mybir as mybir

    uid = 0
    for f in nc.m.functions:
        for b in f.blocks:
            out = []
            changed = False
            for inst in b.instructions:
                si = inst.sync_info
                if si is not None and si.on_wait and len(si.on_wait) > 1:
                    waits = list(si.on_wait)
                    for w in waits[:-1]:
                        uid += 1
                        out.append(mybir.InstNoOp(
                            name=f"I-waitsplit-{uid}",
                            engine=inst.engine,
                            sync_info=mybir.SyncInfo(on_wait=[w], on_update=[]),
                        ))
                    inst.sync_info = mybir.SyncInfo(
                        on_wait=[waits[-1]], on_update=list(si.on_update))
                    changed = True
                out.append(inst)
            if changed:
                b.instructions = out
    return nc


def _dedup_ldweights(nc):
    """Replace a PE Ldweights whose AP+perf_mode exactly match the previous
    Ldweights (with only Matmult instructions between, which don't clobber
    the loaded weights) by a NOP carrying the same sync_info. The following
    Matmult then reuses the already-loaded stationary operand."""
    import concourse.mybir as mybir

    n = 0
    for f in nc.m.functions:
        for b in f.blocks:
            last_sig = None
            out = []
            for inst in b.instructions:
                if inst.engine != mybir.EngineType.PE:
                    out.append(inst)
                    continue
                if inst.opcode == "Ldweights":
                    sig = (str(inst.ins[0]), str(inst.perf_mode))
                    if sig == last_sig:
                        n += 1
                        si = inst.sync_info
                        if si is not None and (si.on_wait or si.on_update):
                            out.append(mybir.InstNoOp(
                                name=f"I-ldwdedup-{n}",
                                engine=inst.engine,
                                sync_info=si,
                            ))
                        continue
                    last_sig = sig
                elif inst.opcode not in ("Matmult", "NoOp"):
                    last_sig = None   # anything else may clobber weights
                out.append(inst)
            b.instructions = out
    return n


def build_nc(tokens=TOK_PER_CORE, loop_n=1, *,
             out_fp16=True,       # y stored as fp16 (exact; host upcasts)
             x_dma="sync",        # HWDGE ring for x loads
             w_dma="scalar",      # ring for w loads
             y_dma="sync",        # ring for y stores (ACT ring would stall
                                  #   ACT's evict work; SP engine is free)
             tevict="split",      # psum_t evictions split ACT/DVE
             yevict="scalar",     # engine for psum_y evictions
             ldw_share=True,      # s-outer/oh-inner matmul order
             xbatch=2,            # token tiles per x load DMA
             quant_mode="dve2",   # dve2 | split2 | dve3 | dve1 (ablation)
             do_transpose=True,   # ablation: False = evict from q directly
             do_mm=True,          # ablation: False = ysb evicted from q
             mm_fd=512,           # ablation: matmul stream FD (512|256)
             mm_oh=2,             # ablation: # of output halves (2|1)
             deep=4,              # SBUF pool bufs for pipeline depth
             psum_split=True,     # per-oh 1-bank psum_y tiles, early evict
             mm_const=False,      # ablation: matmuls read wqT only (free-run)
             ):
    import concourse.bass as bass
    import concourse.mybir as mybir
    from concourse.masks import make_identity
    from concourse.tile import TileContext

    F32 = mybir.dt.float32
    F16 = mybir.dt.float16
    BF16 = mybir.dt.bfloat16
    FP8 = mybir.dt.float8e4
    A = mybir.AluOpType
    YDT = F16 if out_fp16 else F32

    KB = K // P          # 8 k-blocks of 128
    n_ttiles = tokens // P

    nc = bass.Bass()
    x = nc.dram_tensor("x", [tokens, K], F32, kind="ExternalInput")
    w = nc.dram_tensor("weight", [O, K], F32, kind="ExternalInput")
    y = nc.dram_tensor("out", [tokens, O], YDT, kind="ExternalOutput")

    # [t, k] viewed as [t-block, p, k] for batched loads
    x2 = x.rearrange("(a p) k -> a p k", p=P)
    w2 = w.rearrange("(a p) k -> a p k", p=P)
    y2 = y.rearrange("(a p) k -> a p k", p=P)

    def eng(name):
        return {"sync": nc.sync, "scalar": nc.scalar, "vector": nc.vector,
                "gpsimd": nc.gpsimd}[name]

    with TileContext(nc) as tc:
        with (
            tc.tile_pool(name="const", bufs=1) as const_pool,
            tc.tile_pool(name="wqt", bufs=1) as wqt_pool,
            tc.tile_pool(name="win", bufs=2) as win_pool,
            tc.tile_pool(name="xin", bufs=deep) as xin_pool,
            tc.tile_pool(name="quant", bufs=deep) as q_pool,
            tc.tile_pool(name="xqt", bufs=deep) as xqt_pool,
            tc.tile_pool(name="yout", bufs=deep) as y_pool,
            tc.tile_pool(name="psum_t", bufs=2, space="PSUM") as psumt_pool,
            tc.tile_pool(name="psum_y", bufs=6 if psum_split else 3,
                         space="PSUM") as psumy_pool,
        ):
            identity = const_pool.tile([P, P], BF16)
            make_identity(nc, identity)

            def quantize(src):
                """f32 [128, K] view -> NEGATED ternary bf16 [128, K]."""
                q = q_pool.tile([P, K], BF16, tag="q_q")
                if quant_mode == "dve1":      # ablation only (wrong result)
                    nc.vector.tensor_scalar(
                        out=q[:], in0=src, scalar1=THR, scalar2=None,
                        op0=A.is_ge)
                    return q
                if quant_mode in ("split2", "dve2"):
                    u = q_pool.tile([P, K], BF16, tag="q_u")
                    ueng = nc.gpsimd if quant_mode == "split2" else nc.vector
                    ueng.tensor_scalar(
                        out=u[:], in0=src, scalar1=THR, scalar2=None,
                        op0=A.is_ge)
                    # q = (src <= -T) - u  ==  -ternary(src)
                    nc.vector.scalar_tensor_tensor(
                        out=q[:], in0=src, scalar=-THR, in1=u[:],
                        op0=A.is_le, op1=A.subtract)
                    return q
                # dve3: q = -(u - v) = v - u, all on DVE
                u = q_pool.tile([P, K], BF16, tag="q_u")
                nc.vector.tensor_scalar(
                    out=u[:], in0=src, scalar1=THR, scalar2=None, op0=A.is_ge)
                v = q_pool.tile([P, K], BF16, tag="q_v")
                nc.vector.tensor_scalar(
                    out=v[:], in0=src, scalar1=-THR, scalar2=None, op0=A.is_le)
                nc.vector.tensor_tensor(out=q[:], in0=v[:], in1=u[:],
                                        op=A.subtract)
                return q

            def transpose_to(q, dst, evict_engine):
                """q bf16 [128, K] natural -> dst fp8 [128, KB, 128] k-major."""
                if not do_transpose:
                    src = q[:].rearrange("p (b c) -> p b c", b=KB)
                    if evict_engine == "vector":
                        nc.vector.tensor_copy(dst[:], src)
                    else:
                        nc.scalar.copy(dst[:], src)
                    return
                ps = psumt_pool.tile([P, KB, P], BF16, tag="psT")
                for kb in range(KB):
                    nc.tensor.transpose(
                        ps[:, kb, :], q[:, kb * P:(kb + 1) * P], identity)
                if evict_engine == "vector":
                    nc.vector.tensor_copy(dst[:], ps[:])
                elif evict_engine == "split":
                    h = KB // 2
                    nc.scalar.copy(dst[:, :h, :], ps[:, :h, :])
                    nc.vector.tensor_copy(dst[:, h:, :], ps[:, h:, :])
                else:
                    nc.scalar.copy(dst[:], ps[:])

            # --- weight phase: wqT fp8 [k_part, k_blk, o] ---
            wqT = wqt_pool.tile([P, KB, O], FP8)
            for pair in range(O // (2 * P)):       # 4 batched loads
                wt = win_pool.tile([P, 2, K], F32, tag="w_in")
                eng(w_dma).dma_start(
                    wt[:], w2[2 * pair:2 * pair + 2].rearrange("a p k -> p a k"))
                for j in range(2):
                    ob = 2 * pair + j
                    qw = quantize(wt[:, j, :])
                    transpose_to(qw, wqT[:, :, ob * P:(ob + 1) * P], "scalar")

            def do_tile(tb, qx, ysb, j):
                """compute y tile tb from quantized qx, write into ysb[:, j]."""
                xqT = xqt_pool.tile([P, KB, P], FP8, tag="xqT")
                transpose_to(qx, xqT, tevict)

                ydst = ysb[:, j, :]
                if not do_mm:
                    src = qx[:]
                    if yevict == "vector":
                        nc.vector.tensor_copy(ydst, src)
                    else:
                        nc.scalar.copy(ydst, src)
                    return
                def mm(yp_oh, s, oh):
                    lhs = (wqT[:, 2 * s:2 * s + 2, 0:P] if mm_const
                           else xqT[:, 2 * s:2 * s + 2, :])
                    nc.tensor.matmul(
                        yp_oh[:, :mm_fd],
                        lhs,
                        wqT[:, 2 * s:2 * s + 2,
                            oh * 512:oh * 512 + mm_fd],
                        start=(s == 0),
                        stop=(s == KB // 2 - 1),
                        perf_mode=mybir.MatmulPerfMode.DoubleRow,
                    )

                def evict(src, oh):
                    dst = ysb[:, j, oh * 512:(oh + 1) * 512]
                    if yevict == "vector":
                        nc.vector.tensor_copy(dst, src)
                    else:
                        nc.scalar.copy(dst, src)

                if psum_split:
                    if ldw_share:
                        # s-outer with both oh groups live: identical-weight
                        # matmuls adjacent (second Ldweights deduped below)
                        ypA = psumy_pool.tile([P, 512], F32, tag="yp")
                        ypB = psumy_pool.tile([P, 512], F32, tag="yp")
                        yps = [ypA, ypB][:mm_oh]
                        for s in range(KB // 2):
                            for oh in range(mm_oh):
                                mm(yps[oh], s, oh)
                        for oh in range(mm_oh):
                            evict(yps[oh][:], oh)
                    else:
                        # oh-major: finish + evict oh0's bank while oh1 runs
                        for oh in range(mm_oh):
                            yp_oh = psumy_pool.tile([P, 512], F32, tag="yp")
                            for s in range(KB // 2):
                                mm(yp_oh, s, oh)
                            evict(yp_oh[:], oh)
                else:
                    yp = psumy_pool.tile([P, 2, 512], F32, tag="yp")
                    if ldw_share:
                        for s in range(KB // 2):   # 4 DoubleRow k-steps
                            for oh in range(mm_oh):
                                mm(yp[:, oh], s, oh)
                    else:
                        for oh in range(mm_oh):
                            for s in range(KB // 2):
                                mm(yp[:, oh], s, oh)
                    src = yp[:].rearrange("p a b -> p (a b)")
                    if yevict == "vector":
                        nc.vector.tensor_copy(ydst, src)
                    else:
                        nc.scalar.copy(ydst, src)

            def main_body():
                for tp in range(n_ttiles // xbatch):
                    xt = xin_pool.tile([P, xbatch, K], F32, tag="x_in")
                    eng(x_dma).dma_start(
                        xt[:],
                        x2[xbatch * tp:xbatch * (tp + 1)]
                        .rearrange("a p k -> p a k"))
                    ysb = y_pool.tile([P, xbatch, O], YDT, tag="ysb")
                    for j in range(xbatch):
                        tb = xbatch * tp + j
                        qx = quantize(xt[:, j, :])
                        do_tile(tb, qx, ysb, j)
                    eng(y_dma).dma_start(
                        y2[xbatch * tp:xbatch * (tp + 1)]
                        .rearrange("a p k -> p a k"),
                        ysb[:])

            # loop_n > 1 wraps the token loop in a hardware loop purely for
            # benchmarking (amortizes per-call host/PJRT overhead).
            if loop_n > 1:
                with tc.For_i(0, loop_n, 1):
                    main_body()
            else:
                main_body()

    _split_multi_waits(nc)
    _dedup_ldweights(nc)
    return nc


def _get_nc(tokens=TOK_PER_CORE):
    with _lock:
        if tokens not in _cache:
            _cache[tokens] = build_nc(tokens)
        return _cache[tokens]


def kernel(x: np.ndarray, weight: np.ndarray):
    from concourse.bass_utils import run_bass_kernel_spmd

    x = np.ascontiguousarray(x, dtype=np.float32)
    weight = np.ascontiguousarray(weight, dtype=np.float32)
    assert x.shape == (TOKENS, K) and weight.shape == (O, K)

    nc = _get_nc()
    in_maps = [
        {"x": x[i * TOK_PER_CORE:(i + 1) * TOK_PER_CORE], "weight": weight}
        for i in range(N_CORES)
    ]
    res = run_bass_kernel_spmd(nc, in_maps, core_ids=list(range(N_CORES)))
    return np.concatenate(
        [r["out"].astype(np.float32) for r in res.results], axis=0)


# revision 23
# speedup vs baseline: 1.2285x; 1.0102x over previous
"""Ternary (BitwiseLinear) matmul kernel for Trainium2, 8-core data-parallel.

y = ternary(x) @ ternary(w).T  with threshold 0.05, int-exact accumulation.

Sharding: x is split along the token dim across 8 cores (4096 tokens each);
the weight is replicated. Each core computes its y shard independently
(no collectives) and shards are concatenated on the host.

Per-core pipeline (v6):
  1. quantize to NEGATED ternary in 2 DVE ops: u = (v>=T), then
     qneg = (v<=-T) - u (fused scalar_tensor_tensor) -> bf16 {-1,0,1}
     = -ternary(v). Negation applied to BOTH x and w, so the matmul
     product is unchanged. Exact (no element == +-T).
  2. PE-transpose qneg 128x128 blocks into one bf16 PSUM bank; evict
     split across ACT and DVE, casting to fp8e4 -> k-major layout.
  3. fp8 DoubleRow matmuls (K=256 per instruction, s-outer order with
     both 512-wide output halves live in 1-bank PSUM tiles; redundant
     Ldweights NOPed by _dedup_ldweights) accumulate y tile; evict on
     ACT to fp16 (exact: |y| <= 1024 < 2048), DMA out. Host upcasts.
  4. x loads AND y stores on the SP HWDGE ring (SP engine has no compute
     so ring serialization is free); w loads on the ACT ring. y stores
     via ACT/GPSIMD rings measured slower (ACT stalls evicts; SWDGE hits
     a walrus "ISA wrong length" bug inside hardware loops).

Measured loop steady state ~101-110us/iter (32 token tiles; run-to-run
drift +-5us): PE-paced (matmul stream 62us + transposes ~12us + visible
Ldweights/stalls), DMA 24 MiB/iter ~83us, ACT evicts ~47us, DVE ~60us.
"""

import threading

import numpy as np

N_CORES = 8
TOKENS = 32768
TOK_PER_CORE = TOKENS // N_CORES
K = 1024
O = 1024
P = 128
THR = 0.05

_cache = {}
_lock = threading.Lock()


def _split_multi_waits(nc):
    """walrus in this env can't encode >1 sync wait on one instruction: hoist
    extra waits into single-wait NOPs on the same engine, just before the
    instruction (identical per-engine wait semantics)."""
    import concourse.mybir as mybir

    uid = 0
    for f in nc.m.functions:
        for b in f.blocks:
            out = []
            changed = False
            for inst in b.instructions:
                si = inst.sync_info
                if si is not None and si.on_wait and len(si.on_wait) > 1:
                    waits = list(si.on_wait)
                    for w in waits[:-1]:
                        uid += 1
                        out.append(mybir.InstNoOp(
                            name=f"I-waitsplit-{uid}",
                            engine=inst.engine,
                            sync_info=mybir.SyncInfo(on_wait=[w], on_update=[]),
                        ))
                    inst.sync_info = mybir.SyncInfo(
                        on_wait=[waits[-1]], on_update=list(si.on_update))
                    changed = True
                out.append(inst)
            if changed:
                b.instructions = out
    return nc


def _dedup_ldweights(nc):
    """Replace a PE Ldweights whose AP+perf_mode exactly match the previous
    Ldweights (with only Matmult instructions between, which don't clobber
    the loaded weights) by a NOP carrying the same sync_info. The following
    Matmult then reuses the already-loaded stationary operand."""
    import concourse.mybir as mybir

    n = 0
    for f in nc.m.functions:
        for b in f.blocks:
            last_sig = None
            out = []
            for inst in b.instructions:
                if inst.engine != mybir.EngineType.PE:
                    out.append(inst)
                    continue
                if inst.opcode == "Ldweights":
                    sig = (str(inst.ins[0]), str(inst.perf_mode))
                    if sig == last_sig:
                        n += 1
                        si = inst.sync_info
                        if si is not None and (si.on_wait or si.on_update):
                            out.append(mybir.InstNoOp(
                                name=f"I-ldwdedup-{n}",
                                engine=inst.engine,
                                sync_info=si,
                            ))
                        continue
                    last_sig = sig
                elif inst.opcode not in ("Matmult", "NoOp"):
                    last_sig = None   # anything else may clobber weights
                out.append(inst)
            b.instructions = out
    return n


def build_nc(tokens=TOK_PER_CORE, loop_n=1, *,
             out_fp16=True,       # y stored as fp16 (exact; host upcasts)
             x_dma="sync",        # HWDGE ring for x loads
             w_dma="scalar",      # ring for w loads
             y_dma="sync",        # ring for y stores (ACT ring would stall
                                  #   ACT's evict work; SP engine is free)
             tevict="split",      # psum_t evictions split ACT/DVE
             yevict="scalar",     # engine for psum_y evictions
             ldw_share=True,      # s-outer/oh-inner matmul order
             xbatch=2,            # token tiles per x load DMA
             quant_mode="dve2",   # dve2 | split2 | dve3 | dve1 (ablation)
             do_transpose=True,   # ablation: False = evict from q directly
             do_mm=True,          # ablation: False = ysb evicted from q
             mm_fd=512,           # ablation: matmul stream FD (512|256)
             mm_oh=2,             # ablation: # of output halves (2|1)
             deep=4,              # SBUF pool bufs for pipeline depth
             psum_split=True,     # per-oh 1-bank psum_y tiles, early evict
             psum_tb=2,           # psum_t bufs (psum_y gets 8 - psum_tb)
             mm_const=False,      # ablation: matmuls read wqT only (free-run)
             ):
    import concourse.bass as bass
    import concourse.mybir as mybir
    from concourse.masks import make_identity
    from concourse.tile import TileContext

    F32 = mybir.dt.float32
    F16 = mybir.dt.float16
    BF16 = mybir.dt.bfloat16
    FP8 = mybir.dt.float8e4
    A = mybir.AluOpType
    YDT = F16 if out_fp16 else F32

    KB = K // P          # 8 k-blocks of 128
    n_ttiles = tokens // P

    nc = bass.Bass()
    x = nc.dram_tensor("x", [tokens, K], F32, kind="ExternalInput")
    w = nc.dram_tensor("weight", [O, K], F32, kind="ExternalInput")
    y = nc.dram_tensor("out", [tokens, O], YDT, kind="ExternalOutput")

    # [t, k] viewed as [t-block, p, k] for batched loads
    x2 = x.rearrange("(a p) k -> a p k", p=P)
    w2 = w.rearrange("(a p) k -> a p k", p=P)
    y2 = y.rearrange("(a p) k -> a p k", p=P)

    def eng(name):
        return {"sync": nc.sync, "scalar": nc.scalar, "vector": nc.vector,
                "gpsimd": nc.gpsimd}[name]

    with TileContext(nc) as tc:
        with (
            tc.tile_pool(name="const", bufs=1) as const_pool,
            tc.tile_pool(name="wqt", bufs=1) as wqt_pool,
            tc.tile_pool(name="win", bufs=2) as win_pool,
            tc.tile_pool(name="xin", bufs=deep) as xin_pool,
            tc.tile_pool(name="quant", bufs=deep) as q_pool,
            tc.tile_pool(name="xqt", bufs=deep) as xqt_pool,
            tc.tile_pool(name="yout", bufs=deep) as y_pool,
            tc.tile_pool(name="psum_t", bufs=psum_tb, space="PSUM")
                as psumt_pool,
            tc.tile_pool(name="psum_y",
                         bufs=(8 - psum_tb) if psum_split else 3,
                         space="PSUM") as psumy_pool,
        ):
            identity = const_pool.tile([P, P], BF16)
            make_identity(nc, identity)

            def quantize(src):
                """f32 [128, K] view -> NEGATED ternary bf16 [128, K]."""
                q = q_pool.tile([P, K], BF16, tag="q_q")
                if quant_mode == "dve1":      # ablation only (wrong result)
                    nc.vector.tensor_scalar(
                        out=q[:], in0=src, scalar1=THR, scalar2=None,
                        op0=A.is_ge)
                    return q
                if quant_mode in ("split2", "dve2"):
                    u = q_pool.tile([P, K], BF16, tag="q_u")
                    ueng = nc.gpsimd if quant_mode == "split2" else nc.vector
                    ueng.tensor_scalar(
                        out=u[:], in0=src, scalar1=THR, scalar2=None,
                        op0=A.is_ge)
                    # q = (src <= -T) - u  ==  -ternary(src)
                    nc.vector.scalar_tensor_tensor(
                        out=q[:], in0=src, scalar=-THR, in1=u[:],
                        op0=A.is_le, op1=A.subtract)
                    return q
                # dve3: q = -(u - v) = v - u, all on DVE
                u = q_pool.tile([P, K], BF16, tag="q_u")
                nc.vector.tensor_scalar(
                    out=u[:], in0=src, scalar1=THR, scalar2=None, op0=A.is_ge)
                v = q_pool.tile([P, K], BF16, tag="q_v")
                nc.vector.tensor_scalar(
                    out=v[:], in0=src, scalar1=-THR, scalar2=None, op0=A.is_le)
                nc.vector.tensor_tensor(out=q[:], in0=v[:], in1=u[:],
                                        op=A.subtract)
                return q

            def transpose_to(q, dst, evict_engine):
                """q bf16 [128, K] natural -> dst fp8 [128, KB, 128] k-major."""
                if not do_transpose:
                    src = q[:].rearrange("p (b c) -> p b c", b=KB)
                    if evict_engine == "vector":
                        nc.vector.tensor_copy(dst[:], src)
                    else:
                        nc.scalar.copy(dst[:], src)
                    return
                ps = psumt_pool.tile([P, KB, P], BF16, tag="psT")
                for kb in range(KB):
                    nc.tensor.transpose(
                        ps[:, kb, :], q[:, kb * P:(kb + 1) * P], identity)
                if evict_engine == "vector":
                    nc.vector.tensor_copy(dst[:], ps[:])
                elif evict_engine == "split":
                    h = KB // 2
                    nc.scalar.copy(dst[:, :h, :], ps[:, :h, :])
                    nc.vector.tensor_copy(dst[:, h:, :], ps[:, h:, :])
                else:
                    nc.scalar.copy(dst[:], ps[:])

            # --- weight phase: wqT fp8 [k_part, k_blk, o] ---
            wqT = wqt_pool.tile([P, KB, O], FP8)
            for pair in range(O // (2 * P)):       # 4 batched loads
                wt = win_pool.tile([P, 2, K], F32, tag="w_in")
                eng(w_dma).dma_start(
                    wt[:], w2[2 * pair:2 * pair + 2].rearrange("a p k -> p a k"))
                for j in range(2):
                    ob = 2 * pair + j
                    qw = quantize(wt[:, j, :])
                    transpose_to(qw, wqT[:, :, ob * P:(ob + 1) * P], "scalar")

            def do_tile(tb, qx, ysb, j):
                """compute y tile tb from quantized qx, write into ysb[:, j]."""
                xqT = xqt_pool.tile([P, KB, P], FP8, tag="xqT")
                transpose_to(qx, xqT, tevict)

                ydst = ysb[:, j, :]
                if not do_mm:
                    src = qx[:]
                    if yevict == "vector":
                        nc.vector.tensor_copy(ydst, src)
                    else:
                        nc.scalar.copy(ydst, src)
                    return
                def mm(yp_oh, s, oh):
                    lhs = (wqT[:, 2 * s:2 * s + 2, 0:P] if mm_const
                           else xqT[:, 2 * s:2 * s + 2, :])
                    nc.tensor.matmul(
                        yp_oh[:, :mm_fd],
                        lhs,
                        wqT[:, 2 * s:2 * s + 2,
                            oh * 512:oh * 512 + mm_fd],
                        start=(s == 0),
                        stop=(s == KB // 2 - 1),
                        perf_mode=mybir.MatmulPerfMode.DoubleRow,
                    )

                def evict(src, oh):
                    dst = ysb[:, j, oh * 512:(oh + 1) * 512]
                    if yevict == "vector":
                        nc.vector.tensor_copy(dst, src)
                    else:
                        nc.scalar.copy(dst, src)

                if psum_split:
                    if ldw_share:
                        # s-outer with both oh groups live: identical-weight
                        # matmuls adjacent (second Ldweights deduped below)
                        ypA = psumy_pool.tile([P, 512], F32, tag="yp")
                        ypB = psumy_pool.tile([P, 512], F32, tag="yp")
                        yps = [ypA, ypB][:mm_oh]
                        for s in range(KB // 2):
                            for oh in range(mm_oh):
                                mm(yps[oh], s, oh)
                        for oh in range(mm_oh):
                            evict(yps[oh][:], oh)
                    else:
                        # oh-major: finish + evict oh0's bank while oh1 runs
                        for oh in range(mm_oh):
                            yp_oh = psumy_pool.tile([P, 512], F32, tag="yp")
                            for s in range(KB // 2):
                                mm(yp_oh, s, oh)
                            evict(yp_oh[:], oh)
                else:
                    yp = psumy_pool.tile([P, 2, 512], F32, tag="yp")
                    if ldw_share:
                        for s in range(KB // 2):   # 4 DoubleRow k-steps
                            for oh in range(mm_oh):
                                mm(yp[:, oh], s, oh)
                    else:
                        for oh in range(mm_oh):
                            for s in range(KB // 2):
                                mm(yp[:, oh], s, oh)
                    src = yp[:].rearrange("p a b -> p (a b)")
                    if yevict == "vector":
                        nc.vector.tensor_copy(ydst, src)
                    else:
                        nc.scalar.copy(ydst, src)

            def main_body():
                for tp in range(n_ttiles // xbatch):
                    xt = xin_pool.tile([P, xbatch, K], F32, tag="x_in")
                    eng(x_dma).dma_start(
                        xt[:],
                        x2[xbatch * tp:xbatch * (tp + 1)]
                        .rearrange("a p k -> p a k"))
                    ysb = y_pool.tile([P, xbatch, O], YDT, tag="ysb")
                    for j in range(xbatch):
                        tb = xbatch * tp + j
                        qx = quantize(xt[:, j, :])
                        do_tile(tb, qx, ysb, j)
                    eng(y_dma).dma_start(
                        y2[xbatch * tp:xbatch * (tp + 1)]
                        .rearrange("a p k -> p a k"),
                        ysb[:])

            # loop_n > 1 wraps the token loop in a hardware loop purely for
            # benchmarking (amortizes per-call host/PJRT overhead).
            if loop_n > 1:
                with tc.For_i(0, loop_n, 1):
                    main_body()
            else:
                main_body()

    _split_multi_waits(nc)
    _dedup_ldweights(nc)
    return nc


def _get_nc(tokens=TOK_PER_CORE):
    with _lock:
        if tokens not in _cache:
            _cache[tokens] = build_nc(tokens)
        return _cache[tokens]


def kernel(x: np.ndarray, weight: np.ndarray):
    from concourse.bass_utils import run_bass_kernel_spmd

    x = np.ascontiguousarray(x, dtype=np.float32)
    weight = np.ascontiguousarray(weight, dtype=np.float32)
    assert x.shape == (TOKENS, K) and weight.shape == (O, K)

    nc = _get_nc()
    in_maps = [
        {"x": x[i * TOK_PER_CORE:(i + 1) * TOK_PER_CORE], "weight": weight}
        for i in range(N_CORES)
    ]
    res = run_bass_kernel_spmd(nc, in_maps, core_ids=list(range(N_CORES)))
    return np.concatenate(
        [r["out"].astype(np.float32) for r in res.results], axis=0)
